# revision 70
# baseline (speedup 1.0000x reference)
"""Sparse cubic-Bezier Gaussian rasterizer for Trainium2 (Bass/Tile), 8-core.

Math (matches reference.py):
    t = linspace(0, 1, 100);  curve = Bezier3(control_points, t)   # (2, 100)
    gx[t, i] = exp(-(curve_x[t] - i/8192)^2 / 2e-4)                # (100, 8192)
    gy[t, j] = exp(-(curve_y[t] - j/8192)^2 / 2e-4)
    out = gx^T @ gy / 100                                          # (8192, 8192)

The raster is a thin Gaussian tube (sigma = 0.01 = 82 px) around 100 curve
samples; the vast majority of the 8192^2 image is below ~3e-4 (max ~0.039,
||E|| ~ 26.5). Instead of streaming the whole 256 MB image out of HBM, the
host covers every pixel that can exceed a threshold with 512x1024-px tiles
(greedy per-row-band interval cover over a small threshold/row-shift grid;
dropped energy ~2e-4 relative vs the 2e-2 budget) and the device computes
only those tiles into a compact fp16 buffer.

Device pipeline per tile (a 512 x W tile = 4 row-blocks, W in {512,1024}):
  PE:   squared-distance tables as rank-12 bf16 matmuls into PSUM
        (gy: 100 x W, gx: 100 x 512).  d^2 = ramp^2 + 2b*ramp + b^2
        expands over hi/mid/lo bf16 splits of ramp^2, b^2, and the cross
        products, so each table value is exact to ~3e-9 while costing
        1 PE cycle per output column (cost is proportional to the moving
        dim only; contraction depth is free).  b[t] = off_tile - curve[t]
        come in as host inputs (one merged DMA with the constant ramp
        rows).  Then f32r matmuls gx^T @ gy -> (128,512) PSUM banksets.
  ACT:  one Exp per table (PSUM -> SBUF f32r) + ~10/24 of the PSUM->SBUF
        fp16 output copies (the 1/100 normalization rides on the copies'
        scale, so the exps need no bias).
  DVE:  the other output copies.
  DMA:  two 256 KB contiguous half-tile stores per tile (DRAM layout
        [tile][partition][rb][col] matches SBUF partition order); the
        last tile stores per row-block quarter to shrink the drain tail.
Host: scatter tiles into a zeros f32 canvas (clipping overhangs).

K = 3 tiles/core for the canonical input. Cost-model timeline: ~3.0 us
input-DMA fill, ~11.5 us ACT-bound middle (exps + copy share), ~3.9 us
store/semaphore tail -> ~18.3 us vs the 104.3 us dense-fp32 baseline.
"""

import math

import numpy as np

RES = 8192
STEPS = 100
N_CORES = 8
NEG_INV_2SIG = -5000.0  # -1 / 0.0002

TILE_R = 512  # supertile rows (4 row-blocks of 128)
TILE_C = 1024  # supertile cols (one gy table)
RB = TILE_R // 128  # 4
NQ = 12  # rank of the bf16 quadratic expansion
# threshold grid: pick the smallest thr (most accurate) that still
# achieves the minimal tile count
VTHR_GRID = [1e-5, 3e-5, 1e-4, 3e-4, 1e-3]

MM_N = 512  # matmul moving free dim (one PSUM bank of f32)

# tunables: per-rb copy engine patterns, cycled per tile index so the
# ACT:DVE copy split averages 3:5 (ACT also runs the per-tile Exp)
CFG = {
    "copies_cycle": [["dve", "act", "dve", "act", "dve"]],
    "last_copies": ["act", "dve"],
    "merged_slab": False,  # one [gy | gx] PSUM slab + single exp per tile
    "psqy_bufs": 1,
    "psqx_bufs": 1,
    "psmm_bufs": 4,
    "pm_chunk": 512,  # PSUM output tile width (512 -> 1-bank banksets)
    "table_prio": 0,  # how far tables jump ahead of queued output copies
    "pe_warm": 0,  # dummy matmuls ramping PE clock (hurts: in-order queue)
    "obuf_bufs": 3,
    "last_store_quarters": True,  # split last tile's stores to shrink tail
    "last_store_pool": False,  # issue last tile's stores via Pool SWDGE
    "last_store_same_eng": False,  # issue last stores from the copy engine
    "last_rb_chunk_stores": False,  # final row-block streams per chunk
}

_CACHE = {}


def _build_nc(widths):
    """Compile the SPMD program for per-slot tile widths (e.g. (1024, 1024,
    512)); same binary on all 8 cores; the per-tile quadratic coefficients
    arrive as runtime inputs. Narrow slots halve the gy table, the output
    copies, and the stores."""
    import concourse.mybir as mybir
    import concourse.tile as tile
    from concourse import bacc

    f32 = mybir.dt.float32
    f32r = mybir.dt.float32r
    f16 = mybir.dt.float16
    bf16 = mybir.dt.bfloat16
    K = len(widths)
    nc = bacc.Bacc(
        "TRN2", target_bir_lowering=False, debug=False, num_devices=N_CORES
    )

    # one merged input (single DMA): [:, :TILE_C] = constant ramp tables,
    # [:, TILE_C + k*100 : ...] = per-tile y-coeffs, then x-coeffs
    # (see _host_qtab / _host_ls_cols for the 12-row quadratic layout)
    qin_d = nc.dram_tensor(
        "qin", [NQ, TILE_C + 2 * K * STEPS], bf16, kind="ExternalInput"
    )
    # compact output: tile k at rows [k*128, (k+1)*128), laid out
    # [partition][row-block][col] so contiguous DMAs cover the tile.
    out_d = nc.dram_tensor("out", [K * 128, RB * TILE_C], f16, kind="ExternalOutput")

    exp = mybir.ActivationFunctionType.Exp
    mult = mybir.AluOpType.mult

    with tile.TileContext(nc) as tc:
        with (
            tc.tile_pool(name="const", bufs=1) as const,
            tc.tile_pool(name="gyp", bufs=3) as gyp,
            tc.tile_pool(name="gxp", bufs=3) as gxp,
            tc.tile_pool(name="obuf", bufs=CFG["obuf_bufs"]) as obuf,
            tc.tile_pool(name="psqy", bufs=CFG["psqy_bufs"], space="PSUM") as psqy,
            tc.tile_pool(name="psqx", bufs=CFG["psqx_bufs"], space="PSUM") as psqx,
            tc.tile_pool(name="psmm", bufs=CFG["psmm_bufs"], space="PSUM") as psmm,
            tc.tile_pool(name="pswarm", bufs=1, space="PSUM") as pswarm,
        ):
            # ACT Exp table preload via a dummy op.
            warm = const.tile([STEPS, 1], f32)
            nc.vector.memset(warm, 0.0)
            actwarm = const.tile([STEPS, 1], f32)
            nc.scalar.activation(out=actwarm, in_=warm, func=exp)
            # per-partition 1/100 scale for the DVE output copies
            inv_steps = const.tile([128, 1], f32)
            nc.vector.memset(inv_steps, 1.0 / STEPS)

            qin = const.tile([NQ, TILE_C + 2 * K * STEPS], bf16)
            nc.sync.dma_start(out=qin, in_=qin_d.ap())
            qtab = qin[:, :TILE_C]
            ls = qin[:, TILE_C:]

            # PE p-state warm-up: dummy matmuls on zeroed tiles keep the
            # tensor engine busy through the input-DMA latency so the real
            # quad matmuls start at full clock (spare 8th PSUM bank).
            if CFG["pe_warm"]:
                wls = const.tile([NQ, STEPS], bf16)
                nc.vector.memset(wls, 0.0)
                wps = pswarm.tile([STEPS, MM_N], f32)
                wtab = const.tile([NQ, MM_N], bf16)
                nc.vector.memset(wtab, 0.0)
                for _ in range(CFG["pe_warm"]):
                    nc.tensor.matmul(
                        out=wps, lhsT=wls, rhs=wtab, start=True, stop=True
                    )

            for k in range(K):
                W = widths[k]
                cyc = CFG["copies_cycle"]
                copies = (
                    CFG["last_copies"] if k == K - 1 else cyc[k % len(cyc)]
                )
                sly = slice(k * STEPS, (k + 1) * STEPS)
                slx = slice((K + k) * STEPS, (K + k + 1) * STEPS)
                # d^2 tables via rank-12 bf16 matmuls, then Exp off PSUM.
                # High priority: the tables feed everything downstream, so
                # they must not queue behind earlier tiles' output copies
                # (engine queues execute in order).
                with tc.high_priority(CFG["table_prio"]):
                    if CFG["merged_slab"]:
                        # one [gy W | gx 512] PSUM slab, one Exp per tile
                        psy = psqy.tile([STEPS, W + TILE_R], f32, tag="psy")
                        for h in range(0, W, MM_N):
                            nc.tensor.matmul(
                                out=psy[:, h : h + MM_N],
                                lhsT=ls[:, sly],
                                rhs=qtab[:, h : h + MM_N],
                                start=True,
                                stop=True,
                            )
                        nc.tensor.matmul(
                            out=psy[:, W : W + TILE_R],
                            lhsT=ls[:, slx],
                            rhs=qtab[:, 0:TILE_R],
                            start=True,
                            stop=True,
                        )
                        gall = gyp.tile([STEPS, W + TILE_R], f32r, tag="gy")
                        nc.scalar.activation(
                            out=gall, in_=psy, func=exp, scale=NEG_INV_2SIG
                        )
                        gy = gall[:, :W]
                        gx = gall[:, W : W + TILE_R]
                    else:
                        psx = psqx.tile([STEPS, TILE_R], f32, tag="psx")
                        nc.tensor.matmul(
                            out=psx,
                            lhsT=ls[:, slx],
                            rhs=qtab[:, 0:TILE_R],
                            start=True,
                            stop=True,
                        )
                        gx = gxp.tile([STEPS, TILE_R], f32r, tag="gx")
                        nc.scalar.activation(
                            out=gx, in_=psx, func=exp, scale=NEG_INV_2SIG
                        )

                        psy = psqy.tile([STEPS, W], f32, tag="psy")
                        for h in range(0, W, MM_N):
                            nc.tensor.matmul(
                                out=psy[:, h : h + MM_N],
                                lhsT=ls[:, sly],
                                rhs=qtab[:, h : h + MM_N],
                                start=True,
                                stop=True,
                            )
                        gy = gyp.tile([STEPS, W], f32r, tag="gy")
                        nc.scalar.activation(
                            out=gy, in_=psy, func=exp, scale=NEG_INV_2SIG
                        )

                ob = obuf.tile([128, RB * TILE_C], f16, tag="ob")
                pmw = CFG["pm_chunk"]
                for rb in range(RB):
                    lhsT = gx[:, rb * 128 : (rb + 1) * 128]
                    for p0 in range(0, W, pmw):
                        pm = psmm.tile([128, pmw], f32, tag="pm")
                        for h in range(0, pmw, MM_N):
                            nc.tensor.matmul(
                                out=pm[:, h : h + MM_N],
                                lhsT=lhsT,
                                rhs=gy[:, p0 + h : p0 + h + MM_N],
                                start=True,
                                stop=True,
                            )
                        dst = ob[
                            :, rb * TILE_C + p0 : rb * TILE_C + p0 + pmw
                        ]
                        ci = (rb * TILE_C + p0) // pmw
                        # output copy applies the 1/STEPS normalization
                        if copies[ci % len(copies)] == "act":
                            nc.scalar.activation(
                                out=dst,
                                in_=pm,
                                func=mybir.ActivationFunctionType.Copy,
                                scale=1.0 / STEPS,
                            )
                        else:
                            nc.vector.tensor_scalar(
                                out=dst,
                                in0=pm,
                                scalar1=inv_steps,
                                scalar2=None,
                                op0=mult,
                            )
                        # stores launch as soon as their data lands; the
                        # last tile (and any narrow slot) streams per
                        # row-block so the final transfer stays small,
                        # and the very last row-block per chunk
                        if (
                            k == K - 1
                            and rb == RB - 1
                            and CFG["last_rb_chunk_stores"]
                        ):
                            nc.sync.dma_start(
                                out=out_d.ap()[
                                    k * 128 : (k + 1) * 128,
                                    rb * TILE_C
                                    + p0 : rb * TILE_C
                                    + p0
                                    + pmw,
                                ],
                                in_=dst,
                            )
                            continue
                        if p0 + pmw != W:
                            continue
                        if W < TILE_C or (
                            k == K - 1 and CFG["last_store_quarters"]
                        ):
                            if k == K - 1 and CFG["last_store_pool"]:
                                eng = nc.gpsimd
                            elif k == K - 1 and CFG["last_store_same_eng"]:
                                # ACT may initiate DMAs; DVE may not
                                eng = (
                                    nc.scalar
                                    if copies[ci % len(copies)] == "act"
                                    else nc.sync
                                )
                            else:
                                eng = nc.sync
                            eng.dma_start(
                                out=out_d.ap()[
                                    k * 128 : (k + 1) * 128,
                                    rb * TILE_C : rb * TILE_C + W,
                                ],
                                in_=ob[:, rb * TILE_C : rb * TILE_C + W],
                            )
                        elif rb == 1:
                            nc.sync.dma_start(
                                out=out_d.ap()[
                                    k * 128 : (k + 1) * 128, : 2 * TILE_C
                                ],
                                in_=ob[:, : 2 * TILE_C],
                            )
                        elif rb == 3:
                            nc.sync.dma_start(
                                out=out_d.ap()[
                                    k * 128 : (k + 1) * 128, 2 * TILE_C :
                                ],
                                in_=ob[:, 2 * TILE_C :],
                            )

    nc.compile()
    return nc


def _get_nc():
    """nc used by the most recent kernel() call (for TimelineSim in test.py);
    builds a default 3-slot program if kernel() hasn't run yet."""
    if "last_nc" not in _CACHE:
        _CACHE["last_nc"] = _nc_for((1024, 1024, 1024))
    return _CACHE["last_nc"]


def _nc_for(widths):
    widths = tuple(widths)
    if ("nc", widths) not in _CACHE:
        _CACHE[("nc", widths)] = _build_nc(widths)
    nc = _CACHE[("nc", widths)]
    _CACHE["last_nc"] = nc
    return nc


def _basis():
    if "basis" not in _CACHE:
        t = np.linspace(0.0, 1.0, STEPS, dtype=np.float32).astype(np.float64)
        _CACHE["basis"] = np.stack(
            [math.comb(3, k) * (1.0 - t) ** (3 - k) * t**k for k in range(4)]
        )  # (4, STEPS) float64
    return _CACHE["basis"]


def _bf16(x):
    import ml_dtypes

    return np.asarray(x, dtype=np.float32).astype(ml_dtypes.bfloat16)


def _split3(v):
    """v (float64) -> (hi, mid, lo) bf16 cascade with hi+mid+lo ~ v."""
    import ml_dtypes

    h = np.asarray(v, np.float64).astype(ml_dtypes.bfloat16)
    r = v - h.astype(np.float64)
    m = r.astype(ml_dtypes.bfloat16)
    l = (r - m.astype(np.float64)).astype(ml_dtypes.bfloat16)
    return h, m, l


def _host_qtab():
    """Constant rhs rows (12, TILE_C) in bf16.

    d^2[t, c] = ramp(c)^2 + 2 b[t] ramp(c) + b[t]^2 expands to rank 12:
      rows 0-2:  ramp^2 hi/mid/lo      x lhs 1
      rows 3-5:  ones                  x lhs b^2 hi/mid/lo
      rows 6-11: ramp hi,mid,lo combos x lhs 2b hi/mid/lo (see _host_ls)
    """
    if "qtab" not in _CACHE:
        ramp = np.arange(TILE_C, dtype=np.float64) / RES
        r2h, r2m, r2l = _split3(ramp * ramp)
        rh, rm, rl = _split3(ramp)
        one = np.ones_like(ramp)
        rows = [r2h, r2m, r2l, one, one, one, rh, rm, rh, rl, rh, rm]
        _CACHE["qtab"] = np.ascontiguousarray(np.stack([_bf16(r) for r in rows]))
    return _CACHE["qtab"]


def _host_ls_cols(b):
    """lhsT columns (12, 100) for one tile axis given b[t] (float64)."""
    b2h, b2m, b2l = _split3(b * b)
    bh, bm, bl = _split3(b)
    one = np.ones_like(b)
    # pair with qtab rows: [1,1,1, b2h,b2m,b2l, 2bh x rh, 2bh x rm,
    #                       2bm x rh, 2bh x rl, 2bl x rh, 2bm x rm]
    f = np.float64
    rows = [
        one,
        one,
        one,
        b2h.astype(f),
        b2m.astype(f),
        b2l.astype(f),
        2.0 * bh.astype(f),
        2.0 * bh.astype(f),
        2.0 * bm.astype(f),
        2.0 * bh.astype(f),
        2.0 * bl.astype(f),
        2.0 * bm.astype(f),
    ]
    return np.stack([_bf16(r) for r in rows])


def _cover(cx, cy, thr, sr):
    """Greedy cover of all significant pixels with TILE_R x TILE_C tiles:
    rows in bands [i*TILE_R - sr, ...), columns by optimal greedy interval
    cover per band. A pixel can only be significant if some curve sample
    lies within r = sqrt(ln(1/thr)/5000) of it (in unit coords):
    sum_t exp(-5000 d_t^2) <= 100 * exp(-5000 d_min^2) < 100*thr otherwise.
    """
    r = math.sqrt(math.log(1.0 / thr) / 5000.0) * RES
    tiles = []
    nb = (RES + sr + TILE_R - 1) // TILE_R
    for i in range(nb):
        blo = i * TILE_R - sr
        bhi = blo + TILE_R - 1
        blo_c, bhi_c = max(blo, 0), min(bhi, RES - 1)
        if blo_c > bhi_c:
            continue
        dxb = np.maximum(np.maximum(blo_c - cx, cx - bhi_c), 0.0)
        m = dxb <= r
        if not m.any():
            continue
        w = np.sqrt(np.maximum(r * r - dxb[m] ** 2, 0.0))
        los = np.maximum(cy[m] - w, 0.0)
        his = np.minimum(cy[m] + w, RES - 1)
        order = np.argsort(los)
        los, his = los[order], his[order]
        iv = []
        ca, cb = los[0], his[0]
        for a, b in zip(los[1:], his[1:]):
            if a <= cb:
                cb = max(cb, b)
            else:
                iv.append((ca, cb))
                ca, cb = a, b
        iv.append((ca, cb))
        cur_end = -1e18
        cur = None  # (row, col, live_end)
        for a, b in iv:
            if cur is not None and a <= cur_end:
                cur[2] = max(cur[2], min(b, cur[1] + TILE_C))
            x = max(a, cur_end)
            while x <= b:
                start = int(min(x, RES - TILE_C))
                if cur is not None:
                    tiles.append(tuple(cur))
                cur = [blo, start, min(b, start + TILE_C)]
                cur_end = start + TILE_C
                x = cur_end
        if cur is not None:
            tiles.append(tuple(cur))
    # live width rounded up to pm-chunk granularity
    out = []
    for r0, c0, le in tiles:
        w = int(-(-(le - c0 + 1) // 512) * 512)
        out.append((r0, c0, max(512, min(TILE_C, w))))
    return out


def _worklist(cp):
    """Tile worklist covering every pixel that can exceed the threshold.
    Searches a small threshold x row-shift grid; among minimal per-core
    tile counts K, prefers the smallest (most accurate) threshold."""
    curve = _basis().T @ cp.astype(np.float64)  # (100, 2)
    cx, cy = curve[:, 0] * RES, curve[:, 1] * RES

    best = None  # keyed (K, total-chunks, thr_index, n)
    for ti, thr in enumerate(VTHR_GRID):
        for sr in range(0, TILE_R, 64):
            tiles = _cover(cx, cy, thr, sr)
            n = max(len(tiles), 1)
            kk = -(-n // N_CORES)
            nch = sum(w // 512 for _, _, w in tiles)
            key = (kk, nch, ti, n)
            if best is None or key < best[0]:
                best = (key, tiles)
    return best[1]


TRACE = False
LAST_RESULT = None


def kernel(control_points: np.ndarray) -> np.ndarray:
    global LAST_RESULT
    from concourse.bass_utils import run_bass_kernel_spmd

    cp = np.ascontiguousarray(np.asarray(control_points), dtype=np.float32)
    curve = _basis().T @ cp.astype(np.float64)  # (100, 2), float64
    tiles = _worklist(cp)
    canvas = np.zeros((RES, RES), dtype=np.float32)
    if not tiles:
        return canvas

    # sort wide tiles first and pad with (narrow) duplicates so every core
    # gets the same per-slot tile width; the narrowest round runs last so
    # the drain tail is as small as possible
    tiles.sort(key=lambda t: -t[2])
    while len(tiles) % N_CORES:
        tiles.append(tiles[-1])
    K = len(tiles) // N_CORES
    rounds = [tiles[r * N_CORES : (r + 1) * N_CORES] for r in range(K)]
    widths = tuple(max(t[2] for t in rnd) for rnd in rounds)
    percore = [
        [rounds[r][c] for r in range(K)] for c in range(N_CORES)
    ]

    nc = _nc_for(widths)
    qtab = _host_qtab()

    in_maps = []
    for c in range(N_CORES):
        qin = np.empty((NQ, TILE_C + 2 * K * STEPS), qtab.dtype)
        qin[:, :TILE_C] = qtab
        for j, (r0, c0, _w) in enumerate(percore[c]):
            base = TILE_C
            qin[:, base + j * STEPS : base + (j + 1) * STEPS] = _host_ls_cols(
                c0 / RES - curve[:, 1]
            )
            qin[
                :, base + (K + j) * STEPS : base + (K + j + 1) * STEPS
            ] = _host_ls_cols(r0 / RES - curve[:, 0])
        in_maps.append({"qin": np.ascontiguousarray(qin)})

    res = run_bass_kernel_spmd(
        nc, in_maps, core_ids=list(range(N_CORES)), trace=TRACE
    )
    LAST_RESULT = res

    for c in range(N_CORES):
        arr = res.results[c]["out"].reshape(K, 128, RB, TILE_C)
        for j, (r0, c0, w) in enumerate(percore[c]):
            block = arr[j].transpose(1, 0, 2).reshape(TILE_R, TILE_C)
            rs, re = max(r0, 0), min(r0 + TILE_R, RES)
            cs, ce = max(c0, 0), min(c0 + w, RES)
            if rs >= re or cs >= ce:
                continue
            canvas[rs:re, cs:ce] = block[rs - r0 : re - r0, cs - c0 : ce - c0]
    return canvas
